# revision 10
# baseline (speedup 1.0000x reference)
import sys

import ml_dtypes
import numpy as np

_TRN_REPO = "/opt/trn_rl_repo"
if _TRN_REPO not in sys.path:
    sys.path.insert(0, _TRN_REPO)

import concourse.tile as tile
from concourse import bacc, mybir
from concourse.bass_utils import run_bass_kernel_spmd

F32 = mybir.dt.float32
F32R = mybir.dt.float32r
BF16 = mybir.dt.bfloat16
AF = mybir.ActivationFunctionType

B, S, D = 2, 2048, 768
H_TOT, W = 12, 64
N_CORES = 8
HL = 3
DH = HL * W
KC = D // 128
ST = 512
NS = S // ST
PT = 1024
NT = S // 128
BF = ml_dtypes.bfloat16


def _round_f32r(a):
    u = np.ascontiguousarray(a, np.float32).view(np.uint32).copy()
    u += np.uint32(0x7FF) + ((u >> np.uint32(12)) & np.uint32(1))
    u &= np.uint32(0xFFFFF000)
    return u.view(np.float32)


def _emit(tc, aps, has_bias, has_mask):
    nc = tc.nc
    xt_d, wq_d, wk_d, wv_d, on_d, o32_d, mb_d, out_d = aps

    CH = 1 if has_mask else 3
    SCW = CH * ST

    from contextlib import ExitStack

    with ExitStack() as ctx:
        const = ctx.enter_context(tc.tile_pool(name="const", bufs=1))

        ones = None
        if has_bias:
            ones = const.tile([1, PT], BF16, name="ones", tag="ones")
        ones_r = const.tile([1, W], F32R, name="ones_r", tag="ones_r")
        mb = None
        if has_mask:
            mb = const.tile([128, NT], F32, name="mb", tag="mb")

        xt = []
        for c in range(KC):
            t = const.tile([128, S], BF16, name=f"xt{c}", tag=f"xt{c}")
            xt.append(t)
        dmae = [nc.sync, nc.gpsimd, nc.sync]

        def w_tiles(name):
            chunks = []
            for c in range(KC):
                t = const.tile([128, DH], BF16, name=f"{name}{c}",
                               tag=f"{name}{c}")
                chunks.append(t)
            brow = const.tile([1, DH], BF16, name=f"{name}b", tag=f"{name}b")
            return chunks, brow

        wq, wqb = w_tiles("wq")
        wk, wkb = w_tiles("wk")
        wv, wvb = w_tiles("wv")

        for c in range(KC):
            nc.scalar.dma_start(
                out=xt[c][:, 0:ST], in_=xt_d[c * 128:(c + 1) * 128, 0:ST])
            dmae[c % 2].dma_start(
                out=wk[c][:], in_=wk_d[c * 128:(c + 1) * 128, :])
        for c in range(KC):
            dmae[c % 2].dma_start(
                out=wq[c][:], in_=wq_d[c * 128:(c + 1) * 128, :])
        for si in range(1, NS):
            ssl = slice(si * ST, (si + 1) * ST)
            for c in range(KC):
                dmae[(si + c) % 2].dma_start(
                    out=xt[c][:, ssl], in_=xt_d[c * 128:(c + 1) * 128, ssl])
        for c in range(KC):
            dmae[c % 2].dma_start(
                out=wv[c][:], in_=wv_d[c * 128:(c + 1) * 128, :])
        if has_bias:
            for brow, w_d in ((wqb, wq_d), (wkb, wk_d), (wvb, wv_d)):
                nc.sync.dma_start(out=brow[:], in_=w_d[D:D + 1, :])
            nc.sync.dma_start(out=ones[:], in_=on_d[0:1, 0:PT])
        nc.sync.dma_start(out=ones_r[:], in_=o32_d[0:1, :])
        if has_mask:
            nc.sync.dma_start(out=mb[:], in_=mb_d[:, :])

        qt01 = const.tile([128, S], BF16, name="qt01", tag="qt01")
        qt2 = const.tile([128, S], BF16, name="qt2", tag="qt2")
        kt_a = const.tile([128, S], BF16, name="kt_a", tag="kt_a")
        kt_b = const.tile([128, S], BF16, name="kt_b", tag="kt_b")
        vaug = []
        for t in range(NT):
            va = const.tile([128, HL, W + 1], BF16, name=f"vaug{t}",
                            tag=f"vaug{t}")
            nc.gpsimd.memset(va[:, :, W:W + 1], 1.0)
            vaug.append(va)

        strm = ctx.enter_context(
            tc.tile_pool(name="strm", bufs=2, space="PSUM"))
        epi = ctx.enter_context(tc.tile_pool(name="epi", bufs=2))
        exp_pool = ctx.enter_context(tc.tile_pool(name="exp", bufs=6))

        def proj_main(which, si, dst, brow, wch):
            ssl = slice(si * ST, (si + 1) * ST)
            wrk = strm.tile([128, ST], F32, name="wrk", tag="work")
            for c in range(KC):
                nc.tensor.matmul(
                    wrk[:], wch[c][:, 0:128], xt[c][:, ssl],
                    start=(c == 0), stop=False, skip_group_check=True,
                )
            nc.tensor.matmul(
                wrk[:], brow[:, 0:128], ones[:, 0:ST],
                start=False, stop=True, skip_group_check=True,
            )
            nc.vector.tensor_copy(dst[:, ssl], wrk[:])

        def proj_h2_bias(si, wch, brow, dst):
            ssl = slice(si * ST, (si + 1) * ST)
            wrk = strm.tile([128, ST], F32, name="wrkb", tag="work")
            for c in range(KC):
                nc.tensor.matmul(
                    wrk[0:64, :], wch[c][:, 128:DH], xt[c][:, ssl],
                    start=(c == 0), stop=False, skip_group_check=True,
                )
            nc.tensor.matmul(
                wrk[0:64, :], brow[:, 128:DH], ones[:, 0:ST],
                start=False, stop=True, skip_group_check=True,
            )
            nc.vector.tensor_copy(dst[0:64, ssl], wrk[0:64, :])
            nc.vector.tensor_copy(dst[64:128, ssl], wrk[0:64, :])

        if has_bias:
            for si in range(NS):
                proj_main("k", si, kt_a, wkb, wk)
                proj_h2_bias(si, wk, wkb, kt_b)
                proj_main("q", si, qt01, wqb, wq)
                proj_h2_bias(si, wq, wqb, qt2)
            for t in range(NT):
                tsl = slice(t * 128, (t + 1) * 128)
                wrk = strm.tile([128, ST], F32, name="wrkv", tag="work")
                for c in range(KC):
                    nc.tensor.matmul(
                        wrk[:, 0:DH], xt[c][:, tsl], wv[c][:],
                        start=(c == 0), stop=False, skip_group_check=True,
                    )
                nc.tensor.matmul(
                    wrk[:, 0:DH], ones[:, 0:128], wvb[:],
                    start=False, stop=True, skip_group_check=True,
                )
                nc.vector.tensor_copy(
                    vaug[t][:, :, 0:W],
                    wrk[:, 0:DH].rearrange("p (h w) -> p h w", h=HL),
                )

        st_ = {"sc": None, "used": 0, "base": 0, "chunks": []}
        stash = []
        pending = []

        def new_sc():
            st_["sc"] = strm.tile([128, SCW], F32, name="sc", tag="sc")
            st_["used"] = 0
            st_["base"] = 0
            st_["chunks"] = []

        def close_group():
            sc = st_["sc"]
            if sc is None:
                return
            n = len(st_["chunks"])
            if n:
                lo = st_["base"] * ST
                hi = lo + n * ST
                ex = exp_pool.tile([128, SCW], BF16, name="ex", tag="ex")
                tlast = st_["chunks"][-1][2]
                nc.scalar.activation(
                    ex[:, lo:hi], sc[:, lo:hi], AF.Exp,
                    bias=(mb[:, tlast:tlast + 1] if has_mask else 0.0),
                    scale=0.125,
                )
                stash.append(
                    (ex, [(cinfo, h, t, lo + i * ST)
                          for i, (cinfo, h, t) in enumerate(st_["chunks"])]))
            st_["sc"] = None

        def emit_stash(drain=False):
            while len(stash) > (0 if drain else 2):
                ex0, chunks0 = stash.pop(0)
                for (cinfo, h2, t2, col2) in chunks0:
                    nc.tensor.matmul(
                        cinfo["tile"][0:W + 1, :],
                        vaug[t2][:, h2, :],
                        ex0[:, col2:col2 + ST],
                        start=(t2 == 0), stop=(t2 == NT - 1),
                        skip_group_check=True,
                    )
                    cinfo["n"] += 1

        def place_private(nslots):
            close_group()
            out = []
            for _ in range(nslots):
                if st_["sc"] is None or st_["used"] >= CH:
                    close_group()
                    new_sc()
                out.append((st_["sc"], st_["used"] * ST))
                st_["used"] += 1
                st_["base"] = st_["used"]
            if st_["used"] >= CH:
                st_["sc"] = None
            return out

        def place_score():
            if st_["sc"] is None or st_["used"] >= CH:
                close_group()
                new_sc()
            sc, col = st_["sc"], st_["used"] * ST
            st_["used"] += 1
            return sc, col

        def note_score(cinfo, h, t):
            st_["chunks"].append((cinfo, h, t))
            if st_["used"] >= CH:
                close_group()

        def pump():
            emit_stash()

        def epilogue(h, si, ctx_t):
            def run():
                ctx_sb = epi.tile([W, ST], F32R, name="ctx_sb", tag="ctx_sb")
                nc.vector.tensor_copy(ctx_sb[:], ctx_t[0:W, :])
                sumrow = epi.tile([1, ST], F32R, name="sumrow", tag="sumrow")
                nc.vector.tensor_copy(sumrow[:], ctx_t[W:W + 1, :])
                nc.tensor.matmul(
                    ctx_t[0:W, :], ones_r[:], sumrow[:],
                    start=True, stop=True, skip_group_check=True,
                )
                rc = epi.tile([W, ST], F32, name="rc", tag="rc")
                nc.vector.reciprocal_approx_fast(rc[:], ctx_t[0:W, :])
                ot = epi.tile([W, ST], F32, name="ot", tag="ot")
                nc.vector.tensor_mul(ot[:], ctx_sb[:], rc[:])
                nc.sync.dma_start(
                    out=out_d[h * W:(h + 1) * W, si * ST:(si + 1) * ST],
                    in_=ot[:],
                )
            return run

        def flush_ready(force_all=False):
            while pending and (force_all or len(pending) > 2):
                cinfo, fn = pending[0]
                assert cinfo["n"] == NT, "epilogue before ctx done"
                pending.pop(0)
                fn()

        def priv_proj(si, wch, dst):
            ssl = slice(si * ST, (si + 1) * ST)
            [(sc, col)] = place_private(1)
            for c in range(KC):
                nc.tensor.matmul(
                    sc[:, col:col + ST], wch[c][:, 0:128], xt[c][:, ssl],
                    start=(c == 0), stop=(c == KC - 1), skip_group_check=True,
                )
            nc.vector.tensor_copy(dst[:, ssl], sc[:, col:col + ST])

        def priv_q01(si):
            priv_proj(si, wq, qt01)

        def priv_k(si):
            priv_proj(si, wk, kt_a)

        def priv_b4(si):
            ssl = slice(si * ST, (si + 1) * ST)
            (scA, colA), (scB, colB) = place_private(2)
            pa = scA[:, colA:colA + ST]
            pb = scB[:, colB:colB + ST]
            for c in range(KC):
                stt, spp = (c == 0), (c == KC - 1)
                nc.tensor.matmul(
                    pa[0:64, :], wq[c][0:64, 128:DH], xt[c][0:64, ssl],
                    start=stt, stop=spp, skip_group_check=True,
                )
                nc.tensor.matmul(
                    pb[0:64, :], wq[c][64:128, 128:DH], xt[c][64:128, ssl],
                    start=stt, stop=spp, skip_group_check=True,
                )
                nc.tensor.matmul(
                    pa[64:128, :], wk[c][0:64, 128:DH], xt[c][0:64, ssl],
                    start=stt, stop=spp, skip_group_check=True,
                )
                nc.tensor.matmul(
                    pb[64:128, :], wk[c][64:128, 128:DH], xt[c][64:128, ssl],
                    start=stt, stop=spp, skip_group_check=True,
                )
            th = epi.tile([64, ST], F32, name="b4q", tag="b4q")
            nc.vector.tensor_copy(th[:], pa[0:64, :])
            nc.vector.tensor_add(qt2[0:64, ssl], th[:], pb[0:64, :])
            nc.vector.tensor_add(qt2[64:128, ssl], th[:], pb[0:64, :])
            tk = epi.tile([64, ST], F32, name="b4k", tag="b4k")
            nc.vector.tensor_copy(tk[:], pa[64:128, :])
            nc.vector.tensor_add(kt_b[0:64, ssl], tk[:], pb[64:128, :])
            nc.vector.tensor_add(kt_b[64:128, ssl], tk[:], pb[64:128, :])

        def priv_v(t):
            tsl = slice(t * 128, (t + 1) * 128)
            [(sc, col)] = place_private(1)
            pv = sc[:, col:col + DH]
            for c in range(KC):
                nc.tensor.matmul(
                    pv, xt[c][:, tsl], wv[c][:],
                    start=(c == 0), stop=(c == KC - 1), skip_group_check=True,
                )
            nc.vector.tensor_copy(
                vaug[t][:, :, 0:W],
                pv.rearrange("p (h w) -> p h w", h=HL),
            )

        def phase(kind, si, privs):
            qsl = slice(si * ST, (si + 1) * ST)
            if kind == "h01":
                ctx_a = {"tile": strm.tile([128, ST], F32, name="ctxa",
                                           tag="work"), "n": 0}
                ctx_b = {"tile": strm.tile([128, ST], F32, name="ctxb",
                                           tag="work"), "n": 0}
                pairs = [
                    [(kt_a, qt01, 0, t, ctx_a, 0, t),
                     (kt_a, qt01, 1, t, ctx_b, 1, t)]
                    for t in range(NT)
                ]
            else:
                ctx_a = {"tile": strm.tile([128, ST], F32, name="ctxc",
                                           tag="work"), "n": 0}
                ctx_b = None
                pairs = [
                    [(kt_b, qt2, 0, 2 * j, ctx_a, 2, 2 * j),
                     (kt_b, qt2, 1, 2 * j + 1, ctx_a, 2, 2 * j + 1)]
                    for j in range(NT // 2)
                ]
            for i, specs in enumerate(pairs):
                for p in privs.get(i, ()):
                    p()
                for (ktile, qtile, half, t, cinfo, h, tt) in specs:
                    sc, col = place_score()
                    rows = slice(0, 64) if half == 0 else slice(64, 128)
                    nc.tensor.matmul(
                        sc[:, col:col + ST],
                        ktile[rows, t * 128:(t + 1) * 128],
                        qtile[rows, qsl],
                        start=True, stop=True, skip_group_check=True,
                    )
                    note_score(cinfo, h, tt)
                pump()
            close_group()
            emit_stash(drain=True)
            pending.append((ctx_a, epilogue(0 if kind == "h01" else 2,
                                            si, ctx_a["tile"])))
            if ctx_b is not None:
                pending.append((ctx_b, epilogue(1, si, ctx_b["tile"])))
            flush_ready(force_all=True)

        if not has_bias:
            priv_k(0)
            priv_q01(0)
            p0 = {t: [lambda t=t: priv_v(t)] for t in range(NT)}
            p0[1].append(lambda: priv_k(1))
            p0[4].append(lambda: priv_k(2))
            p0[8].append(lambda: priv_k(3))
            phase("h01", 0, p0)
            priv_b4(0)
            priv_q01(1)
            phase("h01", 1, {3: [lambda: priv_b4(1)],
                             9: [lambda: priv_q01(2)]})
            phase("h01", 2, {3: [lambda: priv_b4(2)],
                             9: [lambda: priv_q01(3)]})
            phase("h01", 3, {3: [lambda: priv_b4(3)]})
        else:
            for si in range(NS):
                phase("h01", si, {})
        for si in range(NS):
            phase("h2", si, {})


def _build(has_bias, has_mask):
    nc = bacc.Bacc(
        "TRN2", target_bir_lowering=False, debug=False, num_devices=N_CORES
    )
    xt_d = nc.dram_tensor("xt", [D, S], BF16, kind="ExternalInput").ap()
    wq_d = nc.dram_tensor("wq", [D + 1, DH], BF16, kind="ExternalInput").ap()
    wk_d = nc.dram_tensor("wk", [D + 1, DH], BF16, kind="ExternalInput").ap()
    wv_d = nc.dram_tensor("wv", [D + 1, DH], BF16, kind="ExternalInput").ap()
    on_d = nc.dram_tensor("onesd", [128, PT], BF16, kind="ExternalInput").ap()
    o32_d = nc.dram_tensor("ones32", [1, W], F32R, kind="ExternalInput").ap()
    mb_d = (
        nc.dram_tensor("mb", [128, NT], F32, kind="ExternalInput").ap()
        if has_mask else None
    )
    out_d = nc.dram_tensor("out", [DH, S], F32, kind="ExternalOutput").ap()

    with tile.TileContext(nc) as tc:
        _emit(tc, (xt_d, wq_d, wk_d, wv_d, on_d, o32_d, mb_d, out_d),
              has_bias, has_mask)
    nc.compile()
    return nc


_NC_CACHE = {}


def _get_nc(has_bias, has_mask):
    key = (has_bias, has_mask)
    if key not in _NC_CACHE:
        _NC_CACHE[key] = _build(has_bias, has_mask)
    return _NC_CACHE[key]


def _in_maps(x, Wq, bq, Wk, bk, Wv, bv, mask, has_bias, has_mask):
    xt_by_b = [np.ascontiguousarray(x[b].T).astype(BF) for b in range(B)]
    mb_by_b = [
        np.ascontiguousarray(
            ((np.asarray(mask[b]) == 0).astype(np.float32) * np.float32(-1e30))
            .reshape(NT, 128).T
        )
        for b in range(B)
    ]
    maps = []
    for c in range(N_CORES):
        b, g = divmod(c, N_CORES // B)
        lo = g * DH
        wq_a = np.empty((D + 1, DH), np.float32)
        wq_a[:D] = Wq[lo:lo + DH, :].T
        wq_a[D] = bq[lo:lo + DH]
        wk_a = np.empty((D + 1, DH), np.float32)
        wk_a[:D] = Wk[lo:lo + DH, :].T
        wk_a[D] = bk[lo:lo + DH]
        wv_a = np.empty((D + 1, DH), np.float32)
        wv_a[:D] = Wv[lo:lo + DH, :].T
        wv_a[D] = bv[lo:lo + DH]
        m = {
            "xt": xt_by_b[b], "wq": wq_a.astype(BF), "wk": wk_a.astype(BF),
            "wv": wv_a.astype(BF),
            "onesd": np.ones((128, PT), BF),
            "ones32": _round_f32r(np.ones((1, W), np.float32)),
        }
        if has_mask:
            m["mb"] = mb_by_b[b]
        maps.append(m)
    return maps


def _install_ntff_hook():
    import types

    try:
        from antenv.axon_hooks import get_axon_ntff_profile_hook
        return True
    except ImportError:
        pass
    try:
        import antenv
        from trn_agent_boot.trn_boot import _ntff_profile_via_ctypes

        hook = _ntff_profile_via_ctypes("/opt/axon/libaxon_pjrt.so")
        if hook is None:
            return False
        mod = types.ModuleType("antenv.axon_hooks")
        state = {"hook": hook}
        mod.get_axon_ntff_profile_hook = lambda: state["hook"]
        mod.set_axon_ntff_profile_hook = lambda h: state.update(hook=h)
        sys.modules["antenv.axon_hooks"] = mod
        antenv.axon_hooks = mod
        return True
    except Exception:
        return False


def _run(x, Wq, bq, Wk, bk, Wv, bv, mask, trace=False):
    if trace:
        trace = _install_ntff_hook()
    x = np.ascontiguousarray(np.asarray(x, np.float32))
    Wq = np.asarray(Wq, np.float32)
    Wk = np.asarray(Wk, np.float32)
    Wv = np.asarray(Wv, np.float32)
    bq = np.asarray(bq, np.float32)
    bk = np.asarray(bk, np.float32)
    bv = np.asarray(bv, np.float32)
    has_bias = bool(np.any(bq) or np.any(bk) or np.any(bv))
    has_mask = bool((np.asarray(mask) == 0).any())
    nc = _get_nc(has_bias, has_mask)
    maps = _in_maps(x, Wq, bq, Wk, bk, Wv, bv, mask, has_bias, has_mask)
    res = run_bass_kernel_spmd(nc, maps, list(range(N_CORES)), trace=trace)
    out = np.empty((B, S, D), np.float32)
    for c in range(N_CORES):
        b, g = divmod(c, N_CORES // B)
        out[b, :, g * DH:(g + 1) * DH] = res.results[c]["out"].T
    return out, res


def kernel(x, Wq, bq, Wk, bk, Wv, bv, mask):
    out, _ = _run(x, Wq, bq, Wk, bk, Wv, bv, mask)
    return out


# revision 14
# speedup vs baseline: 1.0864x; 1.0864x over previous
import sys

import ml_dtypes
import numpy as np

_TRN_REPO = "/opt/trn_rl_repo"
if _TRN_REPO not in sys.path:
    sys.path.insert(0, _TRN_REPO)

import concourse.tile as tile
from concourse import bacc, mybir
from concourse.bass_utils import run_bass_kernel_spmd

F32 = mybir.dt.float32
F32R = mybir.dt.float32r
BF16 = mybir.dt.bfloat16
AF = mybir.ActivationFunctionType

B, S, D = 2, 2048, 768
H_TOT, W = 12, 64
N_CORES = 8
HL = 3
DH = HL * W
KC = D // 128
ST = 512
NS = S // ST
PT = 1024
NT = S // 128
BF = ml_dtypes.bfloat16


def _round_f32r(a):
    u = np.ascontiguousarray(a, np.float32).view(np.uint32).copy()
    u += np.uint32(0x7FF) + ((u >> np.uint32(12)) & np.uint32(1))
    u &= np.uint32(0xFFFFF000)
    return u.view(np.float32)


def _emit(tc, aps, has_bias, has_mask):
    nc = tc.nc
    xt_d, wq_d, wk_d, wv_d, on_d, o32_d, mb_d, out_d = aps

    CH = 1 if has_mask else 3
    SCW = CH * ST

    from contextlib import ExitStack

    with ExitStack() as ctx:
        const = ctx.enter_context(tc.tile_pool(name="const", bufs=1))

        ones = None
        if has_bias:
            ones = const.tile([1, PT], BF16, name="ones", tag="ones")
        ones_r = const.tile([1, W], F32R, name="ones_r", tag="ones_r")
        mb = None
        if has_mask:
            mb = const.tile([128, NT], F32, name="mb", tag="mb")

        xt = []
        for c in range(KC):
            t = const.tile([128, S], BF16, name=f"xt{c}", tag=f"xt{c}")
            xt.append(t)
        dmae = [nc.sync, nc.gpsimd, nc.sync]

        def w_tiles(name):
            chunks = []
            for c in range(KC):
                t = const.tile([128, DH], BF16, name=f"{name}{c}",
                               tag=f"{name}{c}")
                chunks.append(t)
            brow = const.tile([1, DH], BF16, name=f"{name}b", tag=f"{name}b")
            return chunks, brow

        wq, wqb = w_tiles("wq")
        wk, wkb = w_tiles("wk")
        wv, wvb = w_tiles("wv")

        for c in range(KC):
            nc.scalar.dma_start(
                out=xt[c][:, 0:ST], in_=xt_d[c * 128:(c + 1) * 128, 0:ST])
            dmae[c % 2].dma_start(
                out=wk[c][:], in_=wk_d[c * 128:(c + 1) * 128, :])
        for c in range(KC):
            dmae[c % 2].dma_start(
                out=wq[c][:], in_=wq_d[c * 128:(c + 1) * 128, :])
        for si in range(1, NS):
            ssl = slice(si * ST, (si + 1) * ST)
            for c in range(KC):
                dmae[(si + c) % 2].dma_start(
                    out=xt[c][:, ssl], in_=xt_d[c * 128:(c + 1) * 128, ssl])
        for c in range(KC):
            dmae[c % 2].dma_start(
                out=wv[c][:], in_=wv_d[c * 128:(c + 1) * 128, :])
        if has_bias:
            for brow, w_d in ((wqb, wq_d), (wkb, wk_d), (wvb, wv_d)):
                nc.sync.dma_start(out=brow[:], in_=w_d[D:D + 1, :])
            nc.sync.dma_start(out=ones[:], in_=on_d[0:1, 0:PT])
        nc.sync.dma_start(out=ones_r[:], in_=o32_d[0:1, :])
        if has_mask:
            nc.sync.dma_start(out=mb[:], in_=mb_d[:, :])

        qt01 = const.tile([128, S], BF16, name="qt01", tag="qt01")
        qt2 = const.tile([128, S], BF16, name="qt2", tag="qt2")
        kt_a = const.tile([128, S], BF16, name="kt_a", tag="kt_a")
        kt_b = const.tile([128, S], BF16, name="kt_b", tag="kt_b")
        vaug = []
        for t in range(NT):
            va = const.tile([128, HL, W + 1], BF16, name=f"vaug{t}",
                            tag=f"vaug{t}")
            nc.gpsimd.memset(va[:, :, W:W + 1], 1.0)
            vaug.append(va)

        strm = ctx.enter_context(
            tc.tile_pool(name="strm", bufs=1, space="PSUM"))
        epi = ctx.enter_context(tc.tile_pool(name="epi", bufs=2))
        exp_pool = ctx.enter_context(tc.tile_pool(name="exp", bufs=6))

        def proj_main(which, si, dst, brow, wch):
            ssl = slice(si * ST, (si + 1) * ST)
            wrk = strm.tile([128, ST], F32, name="wrk", tag="work", bufs=3)
            for c in range(KC):
                nc.tensor.matmul(
                    wrk[:], wch[c][:, 0:128], xt[c][:, ssl],
                    start=(c == 0), stop=False, skip_group_check=True,
                )
            nc.tensor.matmul(
                wrk[:], brow[:, 0:128], ones[:, 0:ST],
                start=False, stop=True, skip_group_check=True,
            )
            nc.vector.tensor_copy(dst[:, ssl], wrk[:])

        def proj_h2_bias(si, wch, brow, dst):
            ssl = slice(si * ST, (si + 1) * ST)
            wrk = strm.tile([128, ST], F32, name="wrkb", tag="work",
                            bufs=3)
            for c in range(KC):
                nc.tensor.matmul(
                    wrk[0:64, :], wch[c][:, 128:DH], xt[c][:, ssl],
                    start=(c == 0), stop=False, skip_group_check=True,
                )
            nc.tensor.matmul(
                wrk[0:64, :], brow[:, 128:DH], ones[:, 0:ST],
                start=False, stop=True, skip_group_check=True,
            )
            nc.vector.tensor_copy(dst[0:64, ssl], wrk[0:64, :])
            nc.vector.tensor_copy(dst[64:128, ssl], wrk[0:64, :])

        if has_bias:
            for si in range(NS):
                proj_main("k", si, kt_a, wkb, wk)
                proj_h2_bias(si, wk, wkb, kt_b)
                proj_main("q", si, qt01, wqb, wq)
                proj_h2_bias(si, wq, wqb, qt2)
            for t in range(NT):
                tsl = slice(t * 128, (t + 1) * 128)
                wrk = strm.tile([128, ST], F32, name="wrkv", tag="work",
                                bufs=3)
                for c in range(KC):
                    nc.tensor.matmul(
                        wrk[:, 0:DH], xt[c][:, tsl], wv[c][:],
                        start=(c == 0), stop=False, skip_group_check=True,
                    )
                nc.tensor.matmul(
                    wrk[:, 0:DH], ones[:, 0:128], wvb[:],
                    start=False, stop=True, skip_group_check=True,
                )
                nc.vector.tensor_copy(
                    vaug[t][:, :, 0:W],
                    wrk[:, 0:DH].rearrange("p (h w) -> p h w", h=HL),
                )

        st_ = {"sc": None, "cap": 0, "used": 0, "base": 0, "chunks": [],
               "which": 0}
        stash = []
        pending = []

        def new_sc():
            w = st_["which"]
            cap = (3, 2)[w] if not has_mask else 1
            st_["sc"] = strm.tile(
                [128, cap * ST], F32, name="sc",
                tag=("scA" if w == 0 else "scB"), bufs=1,
            )
            st_["which"] = 1 - w
            st_["cap"] = cap
            st_["used"] = 0
            st_["base"] = 0
            st_["chunks"] = []

        def close_group():
            sc = st_["sc"]
            if sc is None:
                return
            n = len(st_["chunks"])
            if n:
                lo = st_["base"] * ST
                hi = lo + n * ST
                ex = exp_pool.tile([128, 3 * ST], BF16, name="ex", tag="ex")
                tlast = st_["chunks"][-1][2]
                nc.scalar.activation(
                    ex[:, 0:n * ST], sc[:, lo:hi], AF.Exp,
                    bias=(mb[:, tlast:tlast + 1] if has_mask else 0.0),
                    scale=0.125,
                )
                stash.append(
                    (ex, [(cinfo, h, t, i * ST)
                          for i, (cinfo, h, t) in enumerate(st_["chunks"])]))
            st_["sc"] = None

        def emit_stash(drain=False):
            while len(stash) > (0 if drain else 2):
                ex0, chunks0 = stash.pop(0)
                for (cinfo, h2, t2, col2) in chunks0:
                    nc.tensor.matmul(
                        cinfo["tile"][0:W + 1, :],
                        vaug[t2][:, h2, :],
                        ex0[:, col2:col2 + ST],
                        start=(t2 == 0), stop=(t2 == NT - 1),
                        skip_group_check=True,
                    )
                    cinfo["n"] += 1

        def place_private(nslots):
            close_group()
            out = []
            for _ in range(nslots):
                if st_["sc"] is None or st_["used"] >= st_["cap"]:
                    close_group()
                    new_sc()
                out.append((st_["sc"], st_["used"] * ST))
                st_["used"] += 1
                st_["base"] = st_["used"]
            if st_["used"] >= st_["cap"]:
                st_["sc"] = None
            return out

        def place_score():
            if st_["sc"] is None or st_["used"] >= st_["cap"]:
                close_group()
                new_sc()
            sc, col = st_["sc"], st_["used"] * ST
            st_["used"] += 1
            return sc, col

        def note_score(cinfo, h, t):
            st_["chunks"].append((cinfo, h, t))
            if st_["used"] >= st_["cap"]:
                close_group()

        def pump(allow_p2=True):
            emit_stash()
            if allow_p2 and pending:
                pending.pop(0)()

        def epilogue(h, si, ctx_t):
            ctx_sb = epi.tile([W, ST], F32R, name="ctx_sb", tag="ctx_sb")
            sumrow = epi.tile([1, ST], F32R, name="sumrow", tag="sumrow")

            def p1():
                nc.vector.tensor_copy(sumrow[:], ctx_t[W:W + 1, :])
                nc.vector.tensor_copy(ctx_sb[:], ctx_t[0:W, :])

            def p2():
                nc.tensor.matmul(
                    ctx_t[0:W, :], ones_r[:], sumrow[:],
                    start=True, stop=True, skip_group_check=True,
                )
                rc = epi.tile([W, ST], F32, name="rc", tag="rc")
                nc.vector.reciprocal_approx_fast(rc[:], ctx_t[0:W, :])
                ot = epi.tile([W, ST], F32, name="ot", tag="ot")
                nc.vector.tensor_mul(ot[:], ctx_sb[:], rc[:])
                nc.sync.dma_start(
                    out=out_d[h * W:(h + 1) * W, si * ST:(si + 1) * ST],
                    in_=ot[:],
                )
            return p1, p2

        def priv_proj(si, wch, dst):
            ssl = slice(si * ST, (si + 1) * ST)
            [(sc, col)] = place_private(1)
            for c in range(KC):
                nc.tensor.matmul(
                    sc[:, col:col + ST], wch[c][:, 0:128], xt[c][:, ssl],
                    start=(c == 0), stop=(c == KC - 1), skip_group_check=True,
                )
            nc.vector.tensor_copy(dst[:, ssl], sc[:, col:col + ST])

        def priv_q01(si):
            priv_proj(si, wq, qt01)

        def priv_k(si):
            priv_proj(si, wk, kt_a)

        def priv_b4(si):
            ssl = slice(si * ST, (si + 1) * ST)
            (scA, colA), (scB, colB) = place_private(2)
            pa = scA[:, colA:colA + ST]
            pb = scB[:, colB:colB + ST]
            for c in range(KC):
                stt, spp = (c == 0), (c == KC - 1)
                nc.tensor.matmul(
                    pa[0:64, :], wq[c][0:64, 128:DH], xt[c][0:64, ssl],
                    start=stt, stop=spp, skip_group_check=True,
                )
                nc.tensor.matmul(
                    pb[0:64, :], wq[c][64:128, 128:DH], xt[c][64:128, ssl],
                    start=stt, stop=spp, skip_group_check=True,
                )
                nc.tensor.matmul(
                    pa[64:128, :], wk[c][0:64, 128:DH], xt[c][0:64, ssl],
                    start=stt, stop=spp, skip_group_check=True,
                )
                nc.tensor.matmul(
                    pb[64:128, :], wk[c][64:128, 128:DH], xt[c][64:128, ssl],
                    start=stt, stop=spp, skip_group_check=True,
                )
            th = epi.tile([64, ST], F32, name="b4q", tag="b4q")
            nc.vector.tensor_copy(th[:], pa[0:64, :])
            nc.vector.tensor_add(qt2[0:64, ssl], th[:], pb[0:64, :])
            nc.vector.tensor_add(qt2[64:128, ssl], th[:], pb[0:64, :])
            tk = epi.tile([64, ST], F32, name="b4k", tag="b4k")
            nc.vector.tensor_copy(tk[:], pa[64:128, :])
            nc.vector.tensor_add(kt_b[0:64, ssl], tk[:], pb[64:128, :])
            nc.vector.tensor_add(kt_b[64:128, ssl], tk[:], pb[64:128, :])

        def priv_v(t):
            tsl = slice(t * 128, (t + 1) * 128)
            [(sc, col)] = place_private(1)
            pv = sc[:, col:col + DH]
            for c in range(KC):
                nc.tensor.matmul(
                    pv, xt[c][:, tsl], wv[c][:],
                    start=(c == 0), stop=(c == KC - 1), skip_group_check=True,
                )
            nc.vector.tensor_copy(
                vaug[t][:, :, 0:W],
                pv.rearrange("p (h w) -> p h w", h=HL),
            )

        def phase(kind, si, privs):
            qsl = slice(si * ST, (si + 1) * ST)
            if kind == "h01":
                ctx_a = {"tile": strm.tile([128, ST], F32, name="ctxa",
                                           tag="work", bufs=3), "n": 0}
                ctx_b = {"tile": strm.tile([128, ST], F32, name="ctxb",
                                           tag="work", bufs=3), "n": 0}
                pairs = [
                    [(kt_a, qt01, 0, t, ctx_a, 0, t),
                     (kt_a, qt01, 1, t, ctx_b, 1, t)]
                    for t in range(NT)
                ]
            else:
                ctx_a = {"tile": strm.tile([128, ST], F32, name="ctxc",
                                           tag="work", bufs=3), "n": 0}
                ctx_b = None
                pairs = [
                    [(kt_b, qt2, 0, 2 * j, ctx_a, 2, 2 * j),
                     (kt_b, qt2, 1, 2 * j + 1, ctx_a, 2, 2 * j + 1)]
                    for j in range(NT // 2)
                ]
            for i, specs in enumerate(pairs):
                for p in privs.get(i, ()):
                    p()
                for (ktile, qtile, half, t, cinfo, h, tt) in specs:
                    sc, col = place_score()
                    rows = slice(0, 64) if half == 0 else slice(64, 128)
                    nc.tensor.matmul(
                        sc[:, col:col + ST],
                        ktile[rows, t * 128:(t + 1) * 128],
                        qtile[rows, qsl],
                        start=True, stop=True, skip_group_check=True,
                    )
                    note_score(cinfo, h, tt)
                pump(allow_p2=(i >= 2))
            close_group()
            emit_stash(drain=True)
            ctxs = [(ctx_a, 0 if kind == "h01" else 2)]
            if ctx_b is not None:
                ctxs.append((ctx_b, 1))
            for cinfo, h in ctxs:
                assert cinfo["n"] == NT
                p1, p2 = epilogue(h, si, cinfo["tile"])
                p1()
                pending.append(p2)

        if not has_bias:
            priv_k(0)
            priv_q01(0)
            p0 = {t: [lambda t=t: priv_v(t)] for t in range(NT)}
            p0[1].append(lambda: priv_k(1))
            p0[4].append(lambda: priv_k(2))
            p0[8].append(lambda: priv_k(3))
            phase("h01", 0, p0)
            priv_b4(0)
            priv_q01(1)
            phase("h01", 1, {3: [lambda: priv_b4(1)],
                             9: [lambda: priv_q01(2)]})
            phase("h01", 2, {3: [lambda: priv_b4(2)],
                             9: [lambda: priv_q01(3)]})
            phase("h01", 3, {3: [lambda: priv_b4(3)]})
        else:
            for si in range(NS):
                phase("h01", si, {})
        for si in range(NS):
            phase("h2", si, {})
        while pending:
            pending.pop(0)()


def _build(has_bias, has_mask):
    nc = bacc.Bacc(
        "TRN2", target_bir_lowering=False, debug=False, num_devices=N_CORES
    )
    xt_d = nc.dram_tensor("xt", [D, S], BF16, kind="ExternalInput").ap()
    wq_d = nc.dram_tensor("wq", [D + 1, DH], BF16, kind="ExternalInput").ap()
    wk_d = nc.dram_tensor("wk", [D + 1, DH], BF16, kind="ExternalInput").ap()
    wv_d = nc.dram_tensor("wv", [D + 1, DH], BF16, kind="ExternalInput").ap()
    on_d = nc.dram_tensor("onesd", [128, PT], BF16, kind="ExternalInput").ap()
    o32_d = nc.dram_tensor("ones32", [1, W], F32R, kind="ExternalInput").ap()
    mb_d = (
        nc.dram_tensor("mb", [128, NT], F32, kind="ExternalInput").ap()
        if has_mask else None
    )
    out_d = nc.dram_tensor("out", [DH, S], F32, kind="ExternalOutput").ap()

    with tile.TileContext(nc) as tc:
        _emit(tc, (xt_d, wq_d, wk_d, wv_d, on_d, o32_d, mb_d, out_d),
              has_bias, has_mask)
    nc.compile()
    return nc


_NC_CACHE = {}


def _get_nc(has_bias, has_mask):
    key = (has_bias, has_mask)
    if key not in _NC_CACHE:
        _NC_CACHE[key] = _build(has_bias, has_mask)
    return _NC_CACHE[key]


def _in_maps(x, Wq, bq, Wk, bk, Wv, bv, mask, has_bias, has_mask):
    xt_by_b = [np.ascontiguousarray(x[b].T).astype(BF) for b in range(B)]
    mb_by_b = [
        np.ascontiguousarray(
            ((np.asarray(mask[b]) == 0).astype(np.float32) * np.float32(-1e30))
            .reshape(NT, 128).T
        )
        for b in range(B)
    ]
    maps = []
    for c in range(N_CORES):
        b, g = divmod(c, N_CORES // B)
        lo = g * DH
        wq_a = np.empty((D + 1, DH), np.float32)
        wq_a[:D] = Wq[lo:lo + DH, :].T
        wq_a[D] = bq[lo:lo + DH]
        wk_a = np.empty((D + 1, DH), np.float32)
        wk_a[:D] = Wk[lo:lo + DH, :].T
        wk_a[D] = bk[lo:lo + DH]
        wv_a = np.empty((D + 1, DH), np.float32)
        wv_a[:D] = Wv[lo:lo + DH, :].T
        wv_a[D] = bv[lo:lo + DH]
        m = {
            "xt": xt_by_b[b], "wq": wq_a.astype(BF), "wk": wk_a.astype(BF),
            "wv": wv_a.astype(BF),
            "onesd": np.ones((128, PT), BF),
            "ones32": _round_f32r(np.ones((1, W), np.float32)),
        }
        if has_mask:
            m["mb"] = mb_by_b[b]
        maps.append(m)
    return maps


def _install_ntff_hook():
    import types

    try:
        from antenv.axon_hooks import get_axon_ntff_profile_hook
        return True
    except ImportError:
        pass
    try:
        import antenv
        from trn_agent_boot.trn_boot import _ntff_profile_via_ctypes

        hook = _ntff_profile_via_ctypes("/opt/axon/libaxon_pjrt.so")
        if hook is None:
            return False
        mod = types.ModuleType("antenv.axon_hooks")
        state = {"hook": hook}
        mod.get_axon_ntff_profile_hook = lambda: state["hook"]
        mod.set_axon_ntff_profile_hook = lambda h: state.update(hook=h)
        sys.modules["antenv.axon_hooks"] = mod
        antenv.axon_hooks = mod
        return True
    except Exception:
        return False


def _run(x, Wq, bq, Wk, bk, Wv, bv, mask, trace=False):
    if trace:
        trace = _install_ntff_hook()
    x = np.ascontiguousarray(np.asarray(x, np.float32))
    Wq = np.asarray(Wq, np.float32)
    Wk = np.asarray(Wk, np.float32)
    Wv = np.asarray(Wv, np.float32)
    bq = np.asarray(bq, np.float32)
    bk = np.asarray(bk, np.float32)
    bv = np.asarray(bv, np.float32)
    has_bias = bool(np.any(bq) or np.any(bk) or np.any(bv))
    has_mask = bool((np.asarray(mask) == 0).any())
    nc = _get_nc(has_bias, has_mask)
    maps = _in_maps(x, Wq, bq, Wk, bk, Wv, bv, mask, has_bias, has_mask)
    res = run_bass_kernel_spmd(nc, maps, list(range(N_CORES)), trace=trace)
    out = np.empty((B, S, D), np.float32)
    for c in range(N_CORES):
        b, g = divmod(c, N_CORES // B)
        out[b, :, g * DH:(g + 1) * DH] = res.results[c]["out"].T
    return out, res


def kernel(x, Wq, bq, Wk, bk, Wv, bv, mask):
    out, _ = _run(x, Wq, bq, Wk, bk, Wv, bv, mask)
    return out
